# revision 3
# baseline (speedup 1.0000x reference)
# Trainium2 Bass kernel for the data-reuploading quantum-circuit model
# (nn_DARUAN_45311904972849) — v4: two d-tile chains interleaved per rep.
#
# Same per-op mix as the baseline (which sits at the balanced LP optimum
# of DVE/ACT/Pool pass costs), but the two 128-dim tiles of each batch
# chunk advance through the rep loop together, giving every engine two
# independent dependency chains to hide cross-engine latency.
import sys

sys.path.insert(0, '/opt/trn_rl_repo')
from contextlib import ExitStack

import numpy as np

import concourse.bass as bass  # noqa: F401
import concourse.tile as tile
from concourse import bacc, mybir
from concourse.bass_utils import run_bass_kernel_spmd

AFT = mybir.ActivationFunctionType
ALU = mybir.AluOpType
F32 = mybir.dt.float32
F16 = mybir.dt.float16

from concourse.dve_spec import Spec, Src0, C0, C1, C2, Zero  # noqa: E402
from concourse.dve_ops import DveOp, OPS  # noqa: E402


def _wrap_affine_ref(in0, in1, s0, s1, imm2):
    y = in0 * s0 + s1
    d = y + y
    return y + imm2 * ((d < -imm2).astype(np.float32)
                       - (d > imm2).astype(np.float32))


def _register_wrap_affine():
    for op in OPS:
        if op.name == "WRAP_AFFINE_DARUAN":
            return op
    _y = Src0 * C0 + C1
    _d = _y + _y
    spec = Spec(body=_y + C2 * ((_d < (Zero - C2)) - (_d > C2)),
                reference=_wrap_affine_ref)
    op = DveOp("WRAP_AFFINE_DARUAN", spec, subdim=False,
               uops_sha={"v3": "", "v4": ""})
    OPS.append(op)
    import concourse.dve_ops as _dops
    _dops.CUSTOM_DVE_SPECS[op.name] = op.spec
    _dops._SUB_OPCODE_FOR_NAME[op.name] = (
        _dops._CUSTOM_DVE_ROW_BASE + len(OPS) - 1)
    assert _dops._SUB_OPCODE_FOR_NAME[op.name] < 0x20
    import re as _re
    for ver in ("v3", "v4"):
        try:
            op.compile(ver)
        except ValueError as e:
            m = _re.search(r'="([0-9a-f]{16})"', str(e))
            if not m:
                raise
            op.uops_sha[ver] = m.group(1)
            op.compile(ver)
    return op


WRAP_AFFINE = _register_wrap_affine()

BATCH, DIM, REPS = 4096, 2048, 8
NCORES = 8
DPC = DIM // NCORES
PTILES = DPC // 128
FCH = 2048
BCH = BATCH // FCH
PI = float(np.pi)

_W = 0
_BS = 8
_CT = 16
_ST = 23
_NST = 30
_NXP, _NYP, _NYN, _K1, _K2, _AX, _AZ, _PB, _PI2 = 37, 38, 39, 40, 41, 42, 43, 44, 45
NPARAM = 48

_CACHE = {}


def _build():
    nc = bacc.Bacc('TRN2', target_bir_lowering=False, debug=False,
                   num_devices=NCORES)
    xt_ext = nc.declare_dram_parameter("xt", [DPC, BATCH], F32, isOutput=False)
    pp_ext = nc.declare_dram_parameter("pp", [DPC, NPARAM], F32, isOutput=False)
    yt_ext = nc.declare_dram_parameter("yt", [DPC, BATCH], F32, isOutput=True)

    with ExitStack() as ctx:
        tc = ctx.enter_context(tile.TileContext(nc))
        ppool = ctx.enter_context(tc.tile_pool(name="pp", bufs=1))
        xpool = ctx.enter_context(tc.tile_pool(name="xp", bufs=2))
        apool = ctx.enter_context(tc.tile_pool(name="ang", bufs=2))
        tpool = ctx.enter_context(tc.tile_pool(name="trig", bufs=4))
        spool = ctx.enter_context(tc.tile_pool(name="state", bufs=2))
        mpool = ctx.enter_context(tc.tile_pool(name="tmp", bufs=3))
        opool = ctx.enter_context(tc.tile_pool(name="out", bufs=1))

        # params for both d-tiles stay resident
        pts = []
        for dt in range(PTILES):
            pt = ppool.tile([128, NPARAM], F32, tag=f"pt{dt}")
            nc.sync.dma_start(pt[:], pp_ext[dt * 128:(dt + 1) * 128, :])
            pts.append(pt)

        def col(dt, i):
            return pts[dt][:, i:i + 1]

        for bc in range(BCH):
            xts = []
            for dt in range(PTILES):
                xt = xpool.tile([128, FCH], F32, tag=f"x{dt}")
                nc.sync.dma_start(
                    xt[:], xt_ext[dt * 128:(dt + 1) * 128,
                                  bc * FCH:(bc + 1) * FCH])
                xts.append(xt)

            st = [dict(X=None, Y=None, Z=None) for _ in range(PTILES)]

            def emit_trig(r):
                out = []
                for dt in range(PTILES):
                    US = apool.tile([128, FCH], F16, tag="US")
                    nc.vector._custom_dve(
                        WRAP_AFFINE, out=US[:], in0=xts[dt][:],
                        s0=col(dt, _W + r), s1=col(dt, _BS + r), imm2=2 * PI)
                    UA = apool.tile([128, FCH], F16, tag="UA")
                    nc.scalar.activation(UA[:], US[:], AFT.Abs, bias=0.0,
                                         scale=1.0)
                    S = tpool.tile([128, FCH], F16, tag="S")
                    nc.scalar.activation(S[:], US[:], AFT.Sin, bias=0.0,
                                         scale=1.0)
                    C = tpool.tile([128, FCH], F16, tag="C")
                    nc.scalar.activation(C[:], UA[:], AFT.Sin,
                                         bias=col(dt, _PI2), scale=-1.0)
                    out.append((S, C))
                return out

            trig = emit_trig(0)
            for r in range(REPS):
                next_trig = emit_trig(r + 1) if r + 1 < REPS else None
                for dt in range(PTILES):
                    X, Y, Z = st[dt]["X"], st[dt]["Y"], st[dt]["Z"]
                    S, C = trig[dt]

                    if r == 0:
                        T0 = mpool.tile([128, FCH], F16, tag="M1")
                        nc.vector.tensor_scalar_mul(T0[:], C[:], col(dt, _NXP))
                        T2 = mpool.tile([128, FCH], F16, tag="M3")
                        nc.vector.tensor_scalar_mul(T2[:], S[:], col(dt, _NYN))
                        Xn = spool.tile([128, FCH], F16, tag=f"X{dt}")
                        nc.vector.tensor_add(Xn[:], T0[:], T2[:])
                        T1 = mpool.tile([128, FCH], F16, tag="M2")
                        nc.gpsimd.tensor_scalar(T1[:], C[:], col(dt, _NYP), 0.0,
                                                ALU.mult, ALU.add)
                        T3 = mpool.tile([128, FCH], F16, tag="M4")
                        nc.gpsimd.tensor_scalar(T3[:], S[:], col(dt, _NXP), 0.0,
                                                ALU.mult, ALU.add)
                        Yn = spool.tile([128, FCH], F16, tag=f"Y{dt}")
                        nc.vector.tensor_add(Yn[:], T1[:], T3[:])
                        st[dt]["X"], st[dt]["Y"] = Xn, Yn
                        continue

                    if r == 1:
                        U = mpool.tile([128, FCH], F16, tag="U")
                        nc.vector.tensor_scalar(
                            U[:], X[:], col(dt, _CT), col(dt, _K1),
                            ALU.mult, ALU.add)
                        Zn = spool.tile([128, FCH], F16, tag=f"Z{dt}")
                        nc.scalar.activation(Zn[:], X[:], AFT.Identity,
                                             bias=col(dt, _K2),
                                             scale=col(dt, _NST))
                    else:
                        A = mpool.tile([128, FCH], F16, tag="M1")
                        nc.vector.tensor_scalar_mul(A[:], X[:],
                                                    col(dt, _CT + r - 1))
                        A2 = mpool.tile([128, FCH], F16, tag="M2")
                        nc.vector.tensor_scalar_mul(A2[:], Z[:],
                                                    col(dt, _ST + r - 1))
                        U = mpool.tile([128, FCH], F16, tag="U")
                        nc.vector.tensor_add(U[:], A[:], A2[:])
                        B = mpool.tile([128, FCH], F16, tag="M3")
                        if r == 3:
                            nc.vector.tensor_scalar_mul(
                                B[:], X[:], col(dt, _NST + r - 1))
                        else:
                            nc.scalar.mul(B[:], X[:], col(dt, _NST + r - 1))
                        B2 = mpool.tile([128, FCH], F16, tag="M4")
                        if r == 5:
                            nc.vector.tensor_scalar_mul(
                                B2[:], Z[:], col(dt, _CT + r - 1))
                        else:
                            nc.scalar.mul(B2[:], Z[:], col(dt, _CT + r - 1))
                        Zn = spool.tile([128, FCH], F16, tag=f"Z{dt}")
                        if r == 5:
                            nc.gpsimd.tensor_add(Zn[:], B[:], B2[:])
                        else:
                            nc.vector.tensor_add(Zn[:], B[:], B2[:])

                    M1 = mpool.tile([128, FCH], F16, tag="M1")
                    nc.vector.tensor_mul(M1[:], C[:], U[:])
                    M2 = mpool.tile([128, FCH], F16, tag="M2")
                    nc.gpsimd.tensor_mul(M2[:], S[:], Y[:])
                    Xn = spool.tile([128, FCH], F16, tag=f"X{dt}")
                    nc.vector.tensor_sub(Xn[:], M1[:], M2[:])
                    if r < REPS - 1:
                        M3 = mpool.tile([128, FCH], F16, tag="M3")
                        nc.vector.tensor_mul(M3[:], S[:], U[:])
                        M4 = mpool.tile([128, FCH], F16, tag="M4")
                        nc.gpsimd.tensor_mul(M4[:], C[:], Y[:])
                        Yn = spool.tile([128, FCH], F16, tag=f"Y{dt}")
                        if r == 6:
                            nc.gpsimd.tensor_add(Yn[:], M3[:], M4[:])
                        else:
                            nc.vector.tensor_add(Yn[:], M3[:], M4[:])
                    else:
                        Yn = Y
                    st[dt]["X"], st[dt]["Y"], st[dt]["Z"] = Xn, Yn, Zn
                trig = next_trig

            for dt in range(PTILES):
                O1 = opool.tile([128, FCH], F32, tag="O1")
                nc.scalar.activation(O1[:], st[dt]["X"][:], AFT.Identity,
                                     bias=col(dt, _PB), scale=col(dt, _AX))
                O = opool.tile([128, FCH], F32, tag="O")
                nc.vector.scalar_tensor_tensor(
                    O[:], st[dt]["Z"][:], col(dt, _AZ), O1[:],
                    ALU.mult, ALU.add)
                nc.sync.dma_start(
                    yt_ext[dt * 128:(dt + 1) * 128, bc * FCH:(bc + 1) * FCH],
                    O[:])

    nc.compile()
    return nc


def _fold_params(theta, pw, pb_, ow, ob):
    th = np.asarray(theta, np.float64)
    pw = np.asarray(pw, np.float64)
    pb_ = np.asarray(pb_, np.float64)
    ow = np.asarray(ow, np.float64)
    ob = np.asarray(ob, np.float64)
    t0 = th[:, :REPS, 0]
    t1 = th[:, :REPS, 1]
    tf0 = th[:, REPS, 0]
    tf1 = th[:, REPS, 1]

    P = np.zeros((DIM, NPARAM), np.float64)
    P[:, _W:_W + REPS] = pw
    bs = pb_.copy()
    bs[:, :REPS - 1] += t0[:, 1:]
    bs[:, REPS - 1] += tf0
    P[:, _BS:_BS + REPS] = bs
    ct = np.cos(t1)
    st = np.sin(t1)
    P[:, _CT:_CT + 7] = ct[:, 1:]
    P[:, _ST:_ST + 7] = st[:, 1:]
    P[:, _NST:_NST + 7] = -st[:, 1:]
    nxp = ct[:, 0] * np.cos(t0[:, 0])
    nyp = np.sin(t0[:, 0])
    nzp = -st[:, 0] * np.cos(t0[:, 0])
    P[:, _NXP] = nxp
    P[:, _NYP] = nyp
    P[:, _NYN] = -nyp
    P[:, _K1] = st[:, 1] * nzp
    P[:, _K2] = ct[:, 1] * nzp
    P[:, _AX] = -ow * np.sin(tf1)
    P[:, _AZ] = ow * np.cos(tf1)
    P[:, _PB] = ob
    P[:, _PI2] = np.pi / 2
    return P.astype(np.float32)


def _prep_in_maps(x, theta, preacts_weight, preacts_bias, postact_weights,
                  postact_bias):
    x = np.asarray(x, np.float32)
    P = _fold_params(theta, preacts_weight, preacts_bias, postact_weights,
                     postact_bias)
    in_maps = []
    for c in range(NCORES):
        sl = slice(c * DPC, (c + 1) * DPC)
        in_maps.append({
            "xt": np.ascontiguousarray(x[:, sl].T),
            "pp": np.ascontiguousarray(P[sl]),
        })
    return in_maps


def _gather(results):
    out = np.empty((BATCH, DIM), np.float32)
    for c, r in enumerate(results):
        out[:, c * DPC:(c + 1) * DPC] = r["yt"].T
    return out


def kernel(x, theta, preacts_weight, preacts_bias, postact_weights,
           postact_bias):
    if "nc" not in _CACHE:
        _CACHE["nc"] = _build()
    nc = _CACHE["nc"]
    in_maps = _prep_in_maps(x, theta, preacts_weight, preacts_bias,
                            postact_weights, postact_bias)
    try:
        res = run_bass_kernel_spmd(nc, in_maps, list(range(NCORES)))
    except Exception:
        res = run_bass_kernel_spmd(nc, in_maps, list(range(NCORES)))
    return _gather(res.results)


def run_traced(inputs, trace_cores=None):
    if "nc" not in _CACHE:
        _CACHE["nc"] = _build()
    nc = _CACHE["nc"]
    in_maps = _prep_in_maps(**inputs)
    res = run_bass_kernel_spmd(nc, in_maps, list(range(NCORES)), trace=True,
                               trace_cores=trace_cores)
    return _gather(res.results), res.exec_time_ns
